# revision 1
# baseline (speedup 1.0000x reference)
"""Bahdanau additive attention on 8 Trainium2 NeuronCores.

reference:
    dec_proj = hidden_dec @ Wa                       # [B, U]
    enc_proj = einsum("bse,eu->bsu", outp_enc, Ua)   # [B, S, U]
    e        = tanh(enc_proj + dec_proj[:, None, :])
    scores   = einsum("bsu,u->bs", e, Va)
    alpha    = softmax(scores, axis=-1)
    context  = einsum("bs,bse->be", alpha, outp_enc)  # [B, E]

Sharding: data-parallel on batch. B=32 over 8 cores -> 4 batches/core.
Weights (Wa, Ua, Va) replicated; no collectives.

Per-core plan (4 local batches, S=1024, E=U=D=512), software-pipelined
per batch so softmax/alphaT/context of batch b run under the PE matmuls
of batch b+1:
  - the host ships TWO layouts of the enc shard (layout prep, like the
    transposed hidden state): natural [s,e] in float32r for the context
    matmul, and transposed [e,s] in fp16 (ENCT_DT) for the enc_proj
    matmul -- this removes all on-device transposes + PSUM evacuations.
  - enc_proj in layout [u, s]: lhsT = Ua chunk, rhs = encT; tanh's
    dec_proj bias is a per-partition scalar on the scalar engine.
  - scores = Va . tanh(...) via PE matmul (partition-dim reduction).
  - per-batch softmax on row 32*b (no max-subtraction: |scores| <=
    ||Va||_1 with tanh in (-1,1), far from fp32 exp overflow).
  - alphaT columns via PE transpose of a per-batch alpha tile; context
    via PE with lhsT=alpha column, rhs=enc natural f32r, PSUM-accum.
float32r = full-rate fp32 PE mode (plain fp32 is 4x slower), ~1e-4
relative truncation. fp16 on the scores path adds ~2-4e-4.
"""

import os

import numpy as np

import concourse.bacc as bacc
import concourse.bass as bass
import concourse.mybir as mybir
import concourse.tile as tile
from concourse.bass_utils import run_bass_kernel_spmd
from concourse.masks import make_identity

B, S, E = 32, 1024, 512
D, U = 512, 512
NCORES = 8
BL = B // NCORES          # batches per core
P = 128
EC = E // P               # e chunks (4)
UC = U // P               # u chunks (4)
DC = D // P               # d chunks (4)
ST = S // P               # s 128-chunks per batch (8)
NT = BL * ST              # natural [128, 512] tiles per core (32)

F32 = mybir.dt.float32
F32R = mybir.dt.float32r
ENCT_DT = mybir.dt.float16        # scores-path dtype (fp16 or float32r)
ENCT_NP = np.float16
TANH = mybir.ActivationFunctionType.Tanh
EXP = mybir.ActivationFunctionType.Exp


def build_nc():
    nc = bacc.Bacc("TRN2", target_bir_lowering=False, debug=False,
                   num_devices=NCORES)

    enc = nc.dram_tensor("enc", [BL * S, E], F32R, kind="ExternalInput")
    encT = nc.dram_tensor("encT", [E, BL * S], ENCT_DT, kind="ExternalInput")
    hidT = nc.dram_tensor("hidT", [D, BL], F32R, kind="ExternalInput")
    wa = nc.dram_tensor("wa", [D, U], ENCT_DT, kind="ExternalInput")
    ua = nc.dram_tensor("ua", [E, U], ENCT_DT, kind="ExternalInput")
    va = nc.dram_tensor("va", [UC, P], ENCT_DT, kind="ExternalInput")
    ctx_out = nc.dram_tensor("ctx", [BL, E], F32, kind="ExternalOutput")

    with tile.TileContext(nc) as tc:
        with (
            tc.tile_pool(name="const", bufs=1) as cpool,
            tc.tile_pool(name="nat", bufs=1) as npool,
            tc.tile_pool(name="encT", bufs=1) as tpool,
            tc.tile_pool(name="work", bufs=10) as wpool,
            tc.tile_pool(name="small", bufs=2) as spool,
            tc.tile_pool(name="ps_tr", bufs=2, space="PSUM") as ps_tr,
            tc.tile_pool(name="ps_mm", bufs=3, space="PSUM") as ps_mm,
            tc.tile_pool(name="ps_sc", bufs=3, space="PSUM") as ps_sc,
        ):
            kloop = int(os.environ.get("BASS_ATTN_KLOOP", "1"))
            import contextlib
            loop_cm = tc.For_i(0, kloop, 1) if kloop > 1 else contextlib.nullcontext()
            with loop_cm:
                body(nc, tc, cpool, npool, tpool, wpool, spool,
                     ps_tr, ps_mm, ps_sc, enc, encT, hidT, wa, ua, va, ctx_out)

    nc.compile()
    return nc


def body(nc, tc, cpool, npool, tpool, wpool, spool,
         ps_tr, ps_mm, ps_sc, enc, encT, hidT, wa, ua, va, ctx_out):
            # ---- loads, ordered for the serial DMA stream ----
            eT_sb = tpool.tile([P, EC, BL * S], ENCT_DT)   # [e%128, ec, s]
            encT_r = encT.rearrange("(c p) s -> p c s", p=P)

            def load_encT(b, half=None):
                if half is None:
                    lo = b * S
                    nc.sync.dma_start(out=eT_sb[:, :, lo:lo + S],
                                      in_=encT_r[:, :, lo:lo + S])
                else:
                    lo = b * S + half * 512
                    nc.sync.dma_start(out=eT_sb[:, :, lo:lo + 512],
                                      in_=encT_r[:, :, lo:lo + 512])

            nat = npool.tile([P, NT, E], F32R)
            enc_r = enc.rearrange("(t p) e -> p t e", p=P)

            def load_nat(b):   # 2 MiB granule = one batch's natural tiles
                nc.sync.dma_start(out=nat[:, 8 * b:8 * (b + 1), :],
                                  in_=enc_r[:, 8 * b:8 * (b + 1), :])

            ua_sb = cpool.tile([P, EC, U], ENCT_DT)
            nc.sync.dma_start(out=ua_sb[:], in_=ua.rearrange("(c p) u -> p c u", p=P))
            load_encT(0, 0)
            load_encT(0, 1)
            hidT_sb = cpool.tile([P, DC, BL], F32R)
            nc.sync.dma_start(out=hidT_sb[:], in_=hidT.rearrange("(c p) b -> p c b", p=P))
            va_sb = cpool.tile([P, UC], ENCT_DT)
            nc.sync.dma_start(out=va_sb[:], in_=va.rearrange("c p -> p c"))
            wa_sb = cpool.tile([P, DC, U], ENCT_DT)
            nc.sync.dma_start(out=wa_sb[:], in_=wa.rearrange("(c p) u -> p c u", p=P))
            load_encT(1)
            load_nat(0)
            load_encT(2)
            load_nat(1)
            load_encT(3)
            load_nat(2)
            load_nat(3)

            ident32 = cpool.tile([P, P], F32)
            make_identity(nc, ident32[:])
            ident = cpool.tile([P, P], F32R)
            nc.vector.tensor_copy(ident[:], ident32[:])
            decT_sb = cpool.tile([P, UC, BL], F32)

            hid16 = cpool.tile([P, DC, BL], ENCT_DT)
            nc.vector.tensor_copy(hid16[:], hidT_sb[:])

            def dec_proj():
                # dec_projT[u, b] = sum_d Wa[d, u] * hid[b, d]
                for uc in range(UC):
                    ps = ps_sc.tile([P, BL], F32, tag="sc")
                    for dc in range(DC):
                        nc.tensor.matmul(
                            ps[:], wa_sb[:, dc, uc * P:(uc + 1) * P],
                            hid16[:, dc, :],
                            start=(dc == 0), stop=(dc == DC - 1),
                        )
                    nc.vector.tensor_copy(decT_sb[:, uc, :], ps[:])

            alphas = {}
            esums = {}

            def scores(b, first=False):
                # enc_proj -> tanh for both halves, then all score matmuls
                # (so the score matmuls never wait on a just-issued tanh)
                e_ts = {}
                for half in range(2):
                    sl = slice(b * S + half * 512, b * S + (half + 1) * 512)
                    mm_ps = []
                    for uc in range(UC):
                        psm = ps_mm.tile([P, 512], F32, tag="mm")
                        for ec in range(EC):
                            nc.tensor.matmul(
                                psm[:],
                                ua_sb[:, ec, uc * P:(uc + 1) * P],
                                eT_sb[:, ec, sl],
                                start=(ec == 0), stop=(ec == EC - 1),
                            )
                        mm_ps.append(psm)
                    if first and half == 0:
                        dec_proj()  # Wa arrives right after Ua
                    for uc in range(UC):
                        e_t = wpool.tile([P, 512], ENCT_DT, tag="e")
                        nc.scalar.activation(e_t[:], mm_ps[uc][:], TANH,
                                             bias=decT_sb[:, uc, b:b + 1])
                        e_ts[(half, uc)] = e_t
                alpha = spool.tile([P, S], F32R, tag=f"alpha{b}")
                alphas[b] = alpha
                es0 = spool.tile([P, 1], F32, tag="es0")
                es1 = spool.tile([P, 1], F32, tag="es1")
                esums[b] = [es0, es1]
                r = slice(32 * b, 32 * b + 1)
                for half in range(2):
                    pss = ps_sc.tile([1, 512], F32, tag="sc")
                    for uc in range(UC):
                        nc.tensor.matmul(
                            pss[:], va_sb[:, uc:uc + 1], e_ts[(half, uc)][:],
                            start=(uc == 0), stop=(uc == UC - 1),
                        )
                    osl = slice(half * 512, (half + 1) * 512)
                    # exp straight from the scores PSUM (no evacuation copy)
                    nc.scalar.activation(alpha[r, osl], pss[:], EXP,
                                         accum_out=esums[b][half][r, :])

            rsums = {}

            def softmax(b):
                # alpha stays unnormalized; 1/sum is applied to the final
                # [1, 512] context row instead (off the critical chain).
                r = slice(32 * b, 32 * b + 1)
                ssum = spool.tile([P, 1], F32, tag="ssum")
                nc.vector.tensor_add(ssum[r, :], esums[b][0][r, :],
                                     esums[b][1][r, :])
                rsum = spool.tile([P, 1], F32, tag=f"rsum{b}")
                rsums[b] = rsum
                nc.vector.reciprocal(rsum[r, :], ssum[r, :])

            def context(b):
                # alphaT columns via PE transpose of the per-batch alpha
                # tile (junk rows land in other columns); then ctx matmuls.
                alpha = alphas[b]
                aT = wpool.tile([P, ST], F32R, tag="aT")
                for t in range(ST):
                    psa = ps_tr.tile([P, P], F32R, tag="tr")
                    nc.tensor.transpose(psa[:], alpha[:, t * P:(t + 1) * P],
                                        ident[:])
                    nc.vector.tensor_copy(aT[:, t:t + 1],
                                          psa[:, 32 * b:32 * b + 1])
                psc = ps_sc.tile([1, E], F32, tag="sc")
                for t in range(ST):
                    nc.tensor.matmul(
                        psc[:], aT[:, t:t + 1], nat[:, b * ST + t, :],
                        start=(t == 0), stop=(t == ST - 1),
                    )
                ctx_sb = spool.tile([P, E], F32, tag="ctx")
                r = slice(32 * b, 32 * b + 1)
                nc.scalar.copy(ctx_sb[r, :], psc[:])
                nc.vector.tensor_scalar_mul(ctx_sb[r, :], ctx_sb[r, :],
                                            rsums[b][r, :])
                nc.sync.dma_start(out=ctx_out[b:b + 1, :], in_=ctx_sb[r, :])

            # ---- software pipeline over batches ----
            for b in range(BL):
                scores(b, first=(b == 0))
                softmax(b)
                if b > 0:
                    context(b - 1)
            context(BL - 1)


_NC_CACHE = None


def _in_maps(outp_enc, hidden_dec, Wa, Ua, Va):
    outp_enc = np.ascontiguousarray(outp_enc, dtype=np.float32)
    hidden_dec = np.ascontiguousarray(hidden_dec, dtype=np.float32)
    wa = np.ascontiguousarray(Wa, dtype=ENCT_NP)
    ua = np.ascontiguousarray(Ua, dtype=ENCT_NP)
    va = np.ascontiguousarray(Va, dtype=ENCT_NP).reshape(UC, P)

    in_maps = []
    for c in range(NCORES):
        bs = slice(c * BL, (c + 1) * BL)
        enc_c = outp_enc[bs].reshape(BL * S, E)
        in_maps.append({
            "enc": enc_c,
            "encT": np.ascontiguousarray(enc_c.T.astype(ENCT_NP)),
            "hidT": np.ascontiguousarray(hidden_dec[bs].T),
            "wa": wa, "ua": ua, "va": va,
        })
    return in_maps


def run_spmd(outp_enc, hidden_dec, Wa, Ua, Va, **kwargs):
    global _NC_CACHE
    if _NC_CACHE is None:
        _NC_CACHE = build_nc()
    res = run_bass_kernel_spmd(
        _NC_CACHE, _in_maps(outp_enc, hidden_dec, Wa, Ua, Va),
        core_ids=list(range(NCORES)), **kwargs,
    )
    out = np.concatenate([res.results[c]["ctx"] for c in range(NCORES)], axis=0)
    return out.astype(np.float32), res


def kernel(outp_enc, hidden_dec, Wa, Ua, Va):
    out, _ = run_spmd(outp_enc, hidden_dec, Wa, Ua, Va)
    return out


if __name__ == "__main__":
    rng = np.random.default_rng(0)
    inputs = {
        "outp_enc": rng.standard_normal((B, S, E), dtype=np.float32),
        "hidden_dec": rng.standard_normal((B, D), dtype=np.float32),
        "Wa": (rng.standard_normal((D, U), dtype=np.float32) / np.sqrt(D)),
        "Ua": (rng.standard_normal((E, U), dtype=np.float32) / np.sqrt(E)),
        "Va": (rng.standard_normal((U,), dtype=np.float32) / np.sqrt(U)),
    }
    out = kernel(**inputs)
    print("out", out.shape, out.dtype)

